# revision 16
# baseline (speedup 1.0000x reference)
"""MoE layer (nn_MoELayer_81630148428171) as a Trainium2 Bass kernel on 8 NeuronCores.

Strategy (FF-sharded expert parallelism):
  - Router runs on host (jax-cpu, bitwise-identical ops to the reference).
  - Every core holds a 512-wide slice of the FF dimension of ALL 8 experts
    (w1[:, s*512:(s+1)*512] and w2[s*512:(s+1)*512, :] for slice s = core
    id, 16.8 MB bf16 resident in SBUF) and processes EVERY token chunk of
    every expert — perfect load balance with a single SPMD program whose
    per-core differences are pure data (weight slices; only core 0's b2 is
    nonzero so the bias is added exactly once).
  - Device work per 512-token chunk of expert e:
        hidden^T = gelu(w1_slice[e].T @ xT + b1_slice)   (PE, bf16/f32 acc)
        y_part^T = w2_slice[e].T @ hidden^T (+ b2 on core 0)
    512-wide rhs (one full PSUM bank), gelu on the scalar engine, bias add
    on DVE, both hidden under PE time. The host sums the 8 partial y's
    while unsharding (partials are ~sqrt(8) smaller than y, so the summed
    bf16 rounding noise matches a single rounding).
  - PE streamed-cycle count is the only metric that matters on this part
    (PE is power-throttled to ~1.95GHz; LDWEIGHTS fully hidden), and this
    schedule hits the ideal 16384-token-pair cycle count instead of
    8 x max-expert padding.
  - No transposes, no indirect DMA, no collectives on device.
"""

import math
import numpy as np
import ml_dtypes

import concourse.bacc as bacc
import concourse.mybir as mybir
import concourse.tile as tile
from concourse.bass_utils import run_bass_kernel_spmd

# Problem shapes (hardcoded per contract).
B, SEQ, H = 4, 2048, 1024
T = B * SEQ
FF = 4 * H
E = 8
TOP_K = 2
N_CORES = 8
P = 128

TC = 512               # token-chunk width (= one f32 PSUM bank)
FFS = FF // N_CORES    # per-core FF slice (512)

BF16 = mybir.dt.bfloat16
F32 = mybir.dt.float32
NP_BF16 = ml_dtypes.bfloat16

_PROGRAM_CACHE: dict[tuple, object] = {}


# ----------------------------------------------------------------------------
# Host-side routing + sharding
# ----------------------------------------------------------------------------

def _route(x_flat, router_w, router_b):
    """Top-2 routing with bitwise-identical math to the jax reference."""
    try:
        import jax
        import jax.numpy as jnp

        cpu = jax.devices("cpu")[0]

        def f(xf, w, b):
            logits = xf @ w + b
            probs = jax.nn.softmax(logits, axis=-1)
            top_values, top_indices = jax.lax.top_k(probs, TOP_K)
            top_values = top_values / jnp.sum(top_values, axis=-1,
                                              keepdims=True)
            return top_values, top_indices

        with jax.default_device(cpu):
            tv, ti = jax.jit(f)(
                jnp.asarray(x_flat), jnp.asarray(router_w),
                jnp.asarray(router_b))
        tv = np.asarray(tv)
        ti = np.asarray(ti)
    except Exception:
        # numpy fallback (f32, same tie-breaking as lax.top_k for distinct
        # values — differences only possible for exact float ties)
        logits = x_flat @ router_w + router_b
        p = np.exp(logits - logits.max(-1, keepdims=True))
        p /= p.sum(-1, keepdims=True)
        ti = np.argsort(-p, axis=-1, kind="stable")[:, :TOP_K]
        tv = np.take_along_axis(p, ti, axis=-1)
        tv = tv / tv.sum(-1, keepdims=True)
    return (
        ti[:, 0].astype(np.int64),
        ti[:, 1].astype(np.int64),
        tv[:, 0].astype(np.float32),
        tv[:, 1].astype(np.float32),
    )


# ----------------------------------------------------------------------------
# Device program
# ----------------------------------------------------------------------------

def build_program(all_widths):
    """One SPMD program shared by all 8 cores. `all_widths` is a tuple of
    per-expert chunk-width tuples (compile-time constants, uniform across
    cores). The flattened chunk schedule visits every expert's chunks."""
    sched = [(e, w) for e in range(E) for w in all_widths[e]]
    nch = len(sched)
    act_fn = mybir.ActivationFunctionType.Gelu

    nc = bacc.Bacc("TRN2", target_bir_lowering=False, debug=False,
                   num_devices=N_CORES)

    # All inputs arrive pre-tiled to SBUF layout (host formats them) so every
    # DMA is a fully contiguous per-partition read.
    xg_d = nc.dram_tensor("xg", [nch, P, (H // P) * TC], BF16,
                          kind="ExternalInput")
    # Per-core FF-slice weights, one slab per expert.
    w1_d = nc.dram_tensor("w1b", [E, P, (H // P) * FFS], BF16,
                          kind="ExternalInput")
    w2_d = nc.dram_tensor("w2b", [E, P, (FFS // P) * H], BF16,
                          kind="ExternalInput")
    b1_d = nc.dram_tensor("b1f", [P, E * (FFS // P)], F32,
                          kind="ExternalInput")
    b2_d = nc.dram_tensor("b2f", [P, E * (H // P)], F32,
                          kind="ExternalInput")
    out_d = nc.dram_tensor("out", [nch, P, (H // P) * TC], BF16,
                           kind="ExternalOutput")

    with tile.TileContext(nc) as tc:
        with (
            tc.tile_pool(name="const", bufs=1) as const_pool,
            tc.tile_pool(name="w1", bufs=1) as w1_pool,
            tc.tile_pool(name="w2", bufs=1) as w2_pool,
            tc.tile_pool(name="xg", bufs=2) as xg_pool,
            tc.tile_pool(name="hid", bufs=2) as hid_pool,
            tc.tile_pool(name="yt", bufs=3) as y_pool,
            tc.tile_pool(name="ps", bufs=1, space="PSUM") as ps_pool,
        ):
            # Prologue. Sync ring: w1 slab of the first-scheduled expert
            # k-sliced (first matmul gates on one 0.25MB slice pair), then
            # the other w1 slabs, then the w2 slabs — all loaded ONCE.
            # Scalar ring: xg chunk 0 k-sliced, biases, then later xg
            # chunks (prefetched one chunk ahead).
            xg_first = xg_pool.tile([P, H // P, TC], BF16, tag="xg")
            xg0_src = xg_d[0].rearrange("p (ko s) -> p ko s", ko=H // P)
            w1_tiles = []
            for e in range(E):
                w1t = w1_pool.tile([P, H // P, FFS], BF16, tag=f"w1_{e}")
                w1_src = w1_d[e].rearrange("p (ko m) -> p ko m", ko=H // P)
                if e == 0:
                    nc.scalar.dma_start(out=xg_first[:, 0:1, :],
                                        in_=xg0_src[:, 0:1, :])
                    nc.sync.dma_start(out=w1t[:, 0:1, :],
                                      in_=w1_src[:, 0:1, :])
                    nc.scalar.dma_start(out=xg_first[:, 1:2, :],
                                        in_=xg0_src[:, 1:2, :])
                    nc.sync.dma_start(out=w1t[:, 1:2, :],
                                      in_=w1_src[:, 1:2, :])
                    for j in range(2, H // P, 2):
                        nc.scalar.dma_start(out=xg_first[:, j:j + 2, :],
                                            in_=xg0_src[:, j:j + 2, :])
                        nc.sync.dma_start(out=w1t[:, j:j + 2, :],
                                          in_=w1_src[:, j:j + 2, :])
                else:
                    nc.sync.dma_start(out=w1t[:], in_=w1_src)
                w1_tiles.append(w1t)
            w2_tiles = []
            for e in range(E):
                w2t = w2_pool.tile([P, FFS // P, H], BF16, tag=f"w2_{e}")
                nc.sync.dma_start(
                    out=w2t[:],
                    in_=w2_d[e].rearrange("p (ko n) -> p ko n",
                                          ko=FFS // P))
                w2_tiles.append(w2t)
            # Biases on the scalar HWDGE ring (tiny; no gpsimd/SWDGE in the
            # program, avoiding its per-exec drain).
            b1_sb = const_pool.tile([P, E * (FFS // P)], F32)
            nc.scalar.dma_start(out=b1_sb[:], in_=b1_d[:])
            b2_sb = const_pool.tile([P, E * (H // P)], F32)
            nc.scalar.dma_start(out=b2_sb[:], in_=b2_d[:])

            # Static PSUM banks: 4 for mm1, 4 for mm2.
            pss1 = [ps_pool.tile([P, TC], F32, tag=f"ps1_{i}",
                                 name=f"ps1_{i}") for i in range(4)]
            pss2 = [ps_pool.tile([P, TC], F32, tag=f"ps2_{i}",
                                 name=f"ps2_{i}") for i in range(4)]

            g1 = 0
            g2 = 0
            for c, (e, W) in enumerate(sched):
                if c == 0:
                    xg_sb = xg_first
                else:
                    xg_sb = xg_pool.tile([P, H // P, TC], BF16, tag="xg",
                                         name=f"xg_{c}")
                    nc.scalar.dma_start(
                        out=xg_sb[:],
                        in_=xg_d[c].rearrange("p (ko s) -> p ko s",
                                              ko=H // P))

                # ---- mm1: hidden^T = gelu(w1_slice.T @ xT + b1) ----
                hid = hid_pool.tile([P, FFS // P, TC], BF16, tag="hid",
                                    name=f"hid_{c}")
                if c == 0:
                    # k-outer across the 4 m-tiles: consumes one xg/w1
                    # k-slice per 4 matmuls, matching DMA arrival rate
                    # during the cold start (4 PSUM banks accumulate).
                    for k in range(H // P):
                        for mi in range(FFS // P):
                            nc.tensor.matmul(
                                pss1[mi][:, :W],
                                lhsT=w1_tiles[e][:, k, mi * P:(mi + 1) * P],
                                rhs=xg_sb[:, k, :W],
                                start=(k == 0),
                                stop=(k == H // P - 1),
                            )
                    for mi in range(FFS // P):
                        nc.scalar.activation(
                            hid[:, mi, :W], pss1[mi][:, :W], act_fn,
                            bias=b1_sb[:, e * (FFS // P) + mi:
                                       e * (FFS // P) + mi + 1])
                    g1 = 4
                else:
                    for mi in range(FFS // P):
                        ps = pss1[g1 % 4]
                        g1 += 1
                        for k in range(H // P):
                            nc.tensor.matmul(
                                ps[:, :W],
                                lhsT=w1_tiles[e][:, k, mi * P:(mi + 1) * P],
                                rhs=xg_sb[:, k, :W],
                                start=(k == 0),
                                stop=(k == H // P - 1),
                            )
                        nc.scalar.activation(
                            hid[:, mi, :W], ps[:, :W], act_fn,
                            bias=b1_sb[:, e * (FFS // P) + mi:
                                       e * (FFS // P) + mi + 1])

                # ---- mm2: y_part^T = w2_slice.T @ hidden^T (+ b2) ----
                y_sb = y_pool.tile([P, H // P, TC], BF16, tag="y",
                                   name=f"y_{c}")
                for hi in range(H // P):
                    ps = pss2[g2 % 4]
                    g2 += 1
                    for k in range(FFS // P):
                        nc.tensor.matmul(
                            ps[:, :W],
                            lhsT=w2_tiles[e][:, k, hi * P:(hi + 1) * P],
                            rhs=hid[:, k, :W],
                            start=(k == 0),
                            stop=(k == FFS // P - 1),
                        )
                    nc.vector.tensor_scalar_add(
                        y_sb[:, hi, :W], ps[:, :W],
                        b2_sb[:, e * (H // P) + hi:e * (H // P) + hi + 1])
                if W == TC:
                    half = (H // P) // 2
                    nc.sync.dma_start(out=out_d[c, :, :half * TC],
                                      in_=y_sb[:, :half, :])
                    nc.sync.dma_start(out=out_d[c, :, half * TC:],
                                      in_=y_sb[:, half:, :])
                else:
                    for hx in range(H // P):
                        nc.sync.dma_start(
                            out=out_d[c, :, hx * TC:hx * TC + W],
                            in_=y_sb[:, hx:hx + 1, :W])

    nc.compile()
    return nc


# ----------------------------------------------------------------------------
# Entry point
# ----------------------------------------------------------------------------

def prepare(x, router_w, router_b, w1, b1, w2, b2):
    """Host-side sharding: returns (nc, in_maps, combine meta)."""
    x_flat = np.ascontiguousarray(np.asarray(x, np.float32).reshape(T, H))
    e1, e2, c1, c2 = _route(x_flat, np.asarray(router_w), np.asarray(router_b))

    toks, cvs = [], []
    for e in range(E):
        m1 = e1 == e
        m2 = e2 == e
        toks.append(np.concatenate([np.nonzero(m1)[0], np.nonzero(m2)[0]]))
        cvs.append(np.concatenate([c1[m1], c2[m2]]))
    cnts = [len(t) for t in toks]
    all_widths = []
    for e in range(E):
        cnt = max(cnts[e], 1)
        nch_e = math.ceil(cnt / TC)
        wlast = cnt - (nch_e - 1) * TC
        all_widths.append((TC,) * (nch_e - 1)
                          + (min(TC, (wlast + 1) // 2 * 2),))
    all_widths = tuple(all_widths)

    x_flat_bf = x_flat.astype(NP_BF16)
    w1np = np.asarray(w1, np.float32).astype(NP_BF16)
    w2np = np.asarray(w2, np.float32).astype(NP_BF16)
    b1np = np.asarray(b1, np.float32)
    b2np = np.asarray(b2, np.float32)

    # xg: shared by all cores — every expert's padded chunk grid, in
    # schedule order.
    xg_parts = []
    for e in range(E):
        cntp = len(all_widths[e]) * TC
        xg = np.zeros((H, cntp), NP_BF16)
        xg[:, :cnts[e]] = x_flat_bf[toks[e]].T
        xg_parts.append(
            xg.reshape(H // P, P, cntp // TC, TC).transpose(2, 1, 0, 3)
            .reshape(cntp // TC, P, (H // P) * TC))
    xg_all = np.ascontiguousarray(np.concatenate(xg_parts, axis=0))

    in_maps = []
    for s in range(N_CORES):
        sl = slice(s * FFS, (s + 1) * FFS)
        w1b = np.ascontiguousarray(
            w1np[:, :, sl].reshape(E, H // P, P, FFS).transpose(0, 2, 1, 3)
            .reshape(E, P, (H // P) * FFS))
        w2b = np.ascontiguousarray(
            w2np[:, sl, :].reshape(E, FFS // P, P, H).transpose(0, 2, 1, 3)
            .reshape(E, P, (FFS // P) * H))
        b1f = np.ascontiguousarray(
            b1np[:, sl].reshape(E, FFS // P, P).transpose(2, 0, 1)
            .reshape(P, E * (FFS // P)))
        if s == 0:
            b2f = np.ascontiguousarray(
                b2np.reshape(E, H // P, P).transpose(2, 0, 1)
                .reshape(P, E * (H // P)))
        else:
            b2f = np.zeros((P, E * (H // P)), np.float32)
        in_maps.append(dict(xg=xg_all, w1b=w1b, w2b=w2b, b1f=b1f, b2f=b2f))

    meta = (toks, cvs, cnts, all_widths)
    if all_widths not in _PROGRAM_CACHE:
        _PROGRAM_CACHE[all_widths] = build_program(all_widths)
    return _PROGRAM_CACHE[all_widths], in_maps, meta


def kernel(x, router_w, router_b, w1, b1, w2, b2):
    nc, in_maps, meta = prepare(x, router_w, router_b, w1, b1, w2, b2)
    res = run_bass_kernel_spmd(nc, in_maps, core_ids=list(range(N_CORES)))
    toks, cvs, cnts, all_widths = meta

    # Sum the 8 cores' partial y (f32), then unshard.
    ysum = np.zeros(res.results[0]["out"].shape, np.float32)
    for s in range(N_CORES):
        ysum += np.asarray(res.results[s]["out"]).astype(np.float32)
    nch = ysum.shape[0]
    y_all = (ysum.reshape(nch, P, H // P, TC).transpose(0, 3, 2, 1)
             .reshape(nch * TC, H))

    out_full = np.zeros((T, H), np.float32)
    off = 0
    for e in range(E):
        cntp = len(all_widths[e]) * TC
        y = y_all[off:off + cnts[e]]
        out_full[toks[e]] += cvs[e][:, None] * y
        off += cntp
    return out_full.reshape(B, SEQ, H)
